# revision 10
# baseline (speedup 1.0000x reference)
"""LSTM cell (batch 8192, input 512, hidden 512) on 8 Trainium2 NeuronCores.

Data-parallel over the batch dim: each core handles 1024 rows. Weights are
replicated. The host pre-transposes both matmul operands so the contraction
dim (fan_in = 1024) lands on SBUF partitions:

  gate.T[n, b] = sum_k W.T[k, n] * combined.T[k, b]     (matmul: lhsT.T @ rhs)

so the kernel computes everything in [hidden, batch] layout; gate biases
become per-partition vectors (free on the ACT activation op), and the host
transposes the outputs back after the gather.

Matmul operands are cast to bf16 on the host (f32 matmul on PE is 4x slower);
accumulation is f32 in PSUM. c_prev is loaded and c_next/h_next are stored
as bf16 (rounding ~4e-3 relative, tolerance is 2e-2) to halve the non-matmul
DMA traffic.

v2 schedule notes (from the v1 trace): the 256-matmul stream runs at 216 ns
spacing (ideal 213) once weights are resident, so all the recoverable time
is in the DMA ramp before the first matmul, the DMA-starved first k-tiles,
and the serial tail after the last matmul. Hence:
  - the two tensors the first matmul needs (xh k=0, w k=0/h=0) are the FIRST
    triggers on two different queues (sync, scalar) so their ring cold-starts
    overlap;
  - activations alternate sync/vector so two rings stream them in parallel;
  - weights are batched into one trigger per h-phase (the per-trigger cost
    is ~600 ns of queue time) and sequenced h1 -> h2 -> h3 on gpsimd, each
    landing well before its compute phase;
  - the last (h,b2) group's elementwise tail is split into free-dim chunks
    so the post-matmul serial chain drains in pieces.
"""

import numpy as np

import concourse.bacc as bacc
import concourse.bass as bass
import concourse.mybir as mybir
from concourse import tile
from concourse.bass_utils import run_bass_kernel_spmd

N_CORES = 8
BATCH = 8192
B = BATCH // N_CORES  # 1024 batch rows per core
K = 1024              # fan_in = input_dim + hidden_dim
H = 512               # hidden dim
NG = 4                # gates: i, f, c, o
KT = K // 128         # 8 contraction tiles
HT = H // 128         # 4 hidden chunks per gate
BT = B // 512         # 2 batch halves (PSUM free-dim limit is 512 f32)

MM_DT = mybir.dt.bfloat16
BF16 = mybir.dt.bfloat16
F32 = mybir.dt.float32

_SIG = mybir.ActivationFunctionType.Sigmoid
_TANH = mybir.ActivationFunctionType.Tanh
# gate order within the concatenated weight: i, f, c, o
_GATE_FN = [_SIG, _SIG, _TANH, _SIG]


def _build():
    nc = bacc.Bacc(
        "TRN2",
        target_bir_lowering=False,
        debug=False,
        num_devices=N_CORES,
    )

    # Per-k bundle: activations k-tile (1024 cols) + the h=0 weight strip
    # for that k (512 gate-major cols), side by side, so one trigger per
    # k-tile delivers everything the h=0 phase needs for that k.
    xw = nc.dram_tensor("xw", [KT, 128, B + NG * 128], MM_DT, kind="ExternalInput")
    # Per-h weight block (h>=1) stored as its exact SBUF image: wH[h][p] is
    # the full 8 KB partition row (KT strips of NG*128 gate-major cols), so
    # each phase's load is one trigger with fully contiguous 8 KB elements.
    wH = nc.dram_tensor(
        "wH", [HT - 1, 128, KT, NG * 128], MM_DT, kind="ExternalInput")
    bias2d = nc.dram_tensor("bias2d", [128, NG * HT], F32, kind="ExternalInput")
    c_prevT = nc.dram_tensor("c_prevT", [128, HT, B], BF16, kind="ExternalInput")
    h_nextT = nc.dram_tensor("h_nextT", [H, B], BF16, kind="ExternalOutput")
    c_nextT = nc.dram_tensor("c_nextT", [H, B], BF16, kind="ExternalOutput")

    with tile.TileContext(nc) as tc:
        with (
            tc.tile_pool(name="wts", bufs=1) as wpool,
            tc.tile_pool(name="acts", bufs=1) as apool,
            tc.tile_pool(name="cprev", bufs=1) as cpool,
            tc.tile_pool(name="gates", bufs=3) as gpool,
            tc.tile_pool(name="ew", bufs=3) as epool,
            tc.tile_pool(name="psum", bufs=1, space="PSUM") as pspool,
        ):
            # --- PE warmup ----------------------------------------------
            # The PE runs matmuls ~2x slow until its high-activity boost
            # engages ~4.9 us after the first matmul (HAM record in the
            # trace). A few dummy matmuls on memset garbage, issued while
            # the input DMAs are still in flight, start that clock early
            # so the real stream runs boosted almost from its first op.
            wu_w = epool.tile([128, 128], MM_DT, tag="wu_w", name="wu_w")
            nc.vector.memset(wu_w[:], 0)
            wu_x = epool.tile([128, 512], MM_DT, tag="wu_x", name="wu_x")
            nc.vector.memset(wu_x[:], 0)
            wu_ps = pspool.tile([128, 512], F32, tag="ps3_1", name="wu_ps")
            for _ in range(4):
                nc.tensor.matmul(wu_ps[:], wu_w[:], wu_x[:], start=True, stop=True)

            # --- input DMA schedule -------------------------------------
            # The h=0 phase consumes one (acts + weights) k-bundle per
            # 1.73 us of PE time; each bundle is one 384 KB trigger
            # (~1.1 us of ring time), alternating sync/scalar so the two
            # rings' cold-starts overlap and supply runs ahead of the PE.
            # k=0 is split across both rings so the first matmul's data
            # lands as early as possible. Everything with a later deadline
            # (c_prev, h=2/3 weight phases) rides the TAILS of these same
            # FIFO rings so it cannot steal HBM bandwidth from the
            # h0-critical stream; gpsimd gets only bias + the h=1 weights
            # (needed right when h0 ends) and the mid-kernel output DMAs.
            # Bundle column layout: [acts b2=0 | h0 weights | acts b2=1].
            xw_tiles = [None] * KT

            xt0 = apool.tile([128, B + NG * 128], MM_DT, tag="xw0", name="xw0")
            nc.sync.dma_start(xt0[:, 0:1024], xw[0, :, 0:1024])
            nc.scalar.dma_start(xt0[:, 1024:1536], xw[0, :, 1024:1536])
            xw_tiles[0] = xt0
            for k in range(1, KT):
                xt = apool.tile([128, B + NG * 128], MM_DT, tag=f"xw{k}", name=f"xw{k}")
                (nc.sync if k % 2 == 0 else nc.scalar).dma_start(xt[:], xw[k])
                xw_tiles[k] = xt

            bias_t = wpool.tile([128, NG * HT], F32, tag="bias", name="bias_t")
            nc.gpsimd.dma_start(bias_t[:], bias2d[:])

            wh_tiles = [None] * HT
            for h in range(1, HT):
                wh_tiles[h] = wpool.tile([128, KT, NG * 128], MM_DT,
                                         tag=f"w_h{h}", name=f"w_h{h}")
            nc.gpsimd.dma_start(wh_tiles[1][:], wH[0])

            # late-deadline loads behind the k-bundles (ring-FIFO ordered)
            cp_tile = cpool.tile([128, HT, B], BF16, tag="cp", name="cp")
            nc.sync.dma_start(cp_tile[:], c_prevT[:])
            nc.scalar.dma_start(wh_tiles[2][:], wH[1])
            nc.scalar.dma_start(wh_tiles[3][:], wH[2])

            def _rhs(k, b2):
                off = 0 if b2 == 0 else B // 2 + NG * 128
                return xw_tiles[k][:, off:off + 512]

            def _lhsT(k, h, g):
                if h == 0:
                    return xw_tiles[k][:, 512 + g * 128:512 + (g + 1) * 128]
                return wh_tiles[h][:, k, g * 128:(g + 1) * 128]

            def _mk_psum(g, h, b2):
                return pspool.tile(
                    [128, 512], F32,
                    tag=f"ps{g}_{b2 % 2}", name=f"ps{g}_{h}_{b2}",
                )

            def _elementwise(h, b2, psum, chunks=1, dma_eng=None):
                """Activations + LSTM cell tail for one (h, b2) group.

                chunks>1 splits the free dim so the final group's serial
                ACT->DVE->ACT->DVE chain drains in smaller pieces.
                """
                dma_eng = dma_eng or nc.gpsimd
                hs = slice(h * 128, (h + 1) * 128)
                w = 512 // chunks

                def _act_gate(g, c):
                    t = gpool.tile(
                        [128, w], F32, tag=f"g{g}", name=f"g{g}_{h}_{b2}_{c}",
                    )
                    nc.scalar.activation(
                        t[:], psum[g][:, c * w:(c + 1) * w], _GATE_FN[g],
                        bias=bias_t[:, g * HT + h:g * HT + h + 1],
                    )
                    return t

                for c in range(chunks):
                    cs = slice(b2 * 512 + c * w, b2 * 512 + (c + 1) * w)
                    # i, f, c~ first; the whole c_next/tanh chain runs while
                    # the output gate's matmuls are still on the PE (gate-
                    # major issue order puts o last).
                    gi = _act_gate(0, c)
                    gf = _act_gate(1, c)
                    gc = _act_gate(2, c)

                    t1 = epool.tile([128, w], F32, tag="t1", name=f"t1_{h}_{b2}_{c}")
                    nc.vector.tensor_mul(t1[:], gi[:], gc[:])       # i * c~
                    t2 = epool.tile([128, w], F32, tag="t2", name=f"t2_{h}_{b2}_{c}")
                    nc.vector.tensor_mul(t2[:], gf[:], cp_tile[:, h, cs])
                    cn = epool.tile([128, w], BF16, tag="cn", name=f"cn_{h}_{b2}_{c}")
                    nc.vector.tensor_add(cn[:], t1[:], t2[:])
                    dma_eng.dma_start(c_nextT[hs, cs], cn[:])

                    th = epool.tile([128, w], F32, tag="th", name=f"th_{h}_{b2}_{c}")
                    nc.scalar.activation(th[:], cn[:], _TANH)

                    go = _act_gate(3, c)
                    hn = epool.tile([128, w], BF16, tag="hn", name=f"hn_{h}_{b2}_{c}")
                    nc.vector.tensor_mul(hn[:], go[:], th[:])
                    dma_eng.dma_start(h_nextT[hs, cs], hn[:])

            # h=0 rides the input-DMA ramp: every group needs all 8 k-tiles,
            # so widen to all 8 PSUM banks (4 gates x 2 batch halves) and
            # issue k-major -- the PE consumes each k-tile pair 8 matmuls at
            # a time, right as it lands.
            psum0 = {b2: [_mk_psum(g, 0, b2) for g in range(NG)] for b2 in range(BT)}
            for k in range(KT):
                for g in range(NG):
                    for b2 in range(BT):
                        nc.tensor.matmul(
                            psum0[b2][g][:],
                            _lhsT(k, 0, g),
                            _rhs(k, b2),
                            start=(k == 0),
                            stop=(k == KT - 1),
                        )
            for b2 in range(BT):
                _elementwise(0, b2, psum0[b2])

            # h>=1: inputs are resident; per-(h,b2) 4-bank groups with b2
            # parity alternating between the two bank sets, so each set's
            # ACT drain overlaps the other's matmuls.
            for h in range(1, HT):
                for b2 in range(BT):
                    psum = [_mk_psum(g, h, b2) for g in range(NG)]
                    # gate-major, output gate (g=3) last: everything except
                    # ACT(o) and h=o*tanh(c) drains while o's matmuls run.
                    for g in range(NG):
                        for k in range(KT):
                            nc.tensor.matmul(
                                psum[g][:],
                                _lhsT(k, h, g),
                                _rhs(k, b2),
                                start=(k == 0),
                                stop=(k == KT - 1),
                            )
                    last = (h == HT - 1 and b2 == BT - 1)
                    _elementwise(
                        h, b2, psum,
                        chunks=2 if last else 1,
                        dma_eng=nc.sync if last else None,
                    )

    nc.compile()
    return nc


_NC_CACHE = None
_LAST_IN_MAPS = None


def kernel(x, h_prev, c_prev, W_i, b_i, W_f, b_f, W_c, b_c, W_o, b_o):
    global _NC_CACHE, _LAST_IN_MAPS
    if _NC_CACHE is None:
        _NC_CACHE = _build()
    nc = _NC_CACHE

    np_bf16 = mybir.dt.np(MM_DT)

    combT = np.concatenate([x, h_prev], axis=1).T          # (K, BATCH) f32
    combT = combT.astype(np_bf16)
    wT = np.concatenate([W_i, W_f, W_c, W_o], axis=0).T    # (K, 4H): col g*H+h*128+p
    # wHk[h][p][k] = 512 gate-major cols of strip (k, h) for partition p
    wHk = np.ascontiguousarray(
        wT.reshape(KT, 128, NG, HT, 128).transpose(3, 1, 0, 2, 4)
        .reshape(HT, 128, KT, NG * 128)
    ).astype(np_bf16)
    bias2d = np.ascontiguousarray(
        np.concatenate([b_i, b_f, b_c, b_o]).reshape(NG * HT, 128).T
    ).astype(np.float32)                                   # (128, 16)
    c_prevT = c_prev.T.astype(np_bf16)                     # (H, BATCH) bf16

    in_maps = []
    for j in range(N_CORES):
        cols = slice(j * B, (j + 1) * B)
        # per-k bundle: [acts b2=0 | h=0 weight strip | acts b2=1]
        acts = combT[:, cols].reshape(KT, 128, B)
        xwj = np.concatenate(
            [acts[:, :, :512], wHk[0].transpose(1, 0, 2), acts[:, :, 512:]],
            axis=2)
        in_maps.append({
            "xw": np.ascontiguousarray(xwj),
            "wH": wHk[1:],
            "bias2d": bias2d,
            "c_prevT": np.ascontiguousarray(
                c_prevT[:, cols].reshape(HT, 128, B).transpose(1, 0, 2)
            ),
        })

    _LAST_IN_MAPS = in_maps
    try:
        res = run_bass_kernel_spmd(nc, in_maps, core_ids=list(range(N_CORES)))
    except Exception:
        # transient NRT_EXEC_UNIT_UNRECOVERABLE has been observed once on an
        # otherwise-correct NEFF; one retry is cheap insurance.
        res = run_bass_kernel_spmd(nc, in_maps, core_ids=list(range(N_CORES)))

    h_next = np.concatenate(
        [r["h_nextT"].astype(np.float32).T for r in res.results], axis=0)
    c_next = np.concatenate(
        [r["c_nextT"].astype(np.float32).T for r in res.results], axis=0)
    return (h_next, c_next)


# revision 11
# speedup vs baseline: 1.0263x; 1.0263x over previous
"""LSTM cell (batch 8192, input 512, hidden 512) on 8 Trainium2 NeuronCores.

Data-parallel over the batch dim: each core handles 1024 rows. Weights are
replicated. The host pre-transposes both matmul operands so the contraction
dim (fan_in = 1024) lands on SBUF partitions:

  gate.T[n, b] = sum_k W.T[k, n] * combined.T[k, b]     (matmul: lhsT.T @ rhs)

so the kernel computes everything in [hidden, batch] layout; gate biases
become per-partition vectors (free on the ACT activation op), and the host
transposes the outputs back after the gather.

Matmul operands are cast to bf16 on the host (f32 matmul on PE is 4x slower);
accumulation is f32 in PSUM. c_prev is loaded and c_next/h_next are stored
as bf16 (rounding ~4e-3 relative, tolerance is 2e-2) to halve the non-matmul
DMA traffic.

v2 schedule notes (from the v1 trace): the 256-matmul stream runs at 216 ns
spacing (ideal 213) once weights are resident, so all the recoverable time
is in the DMA ramp before the first matmul, the DMA-starved first k-tiles,
and the serial tail after the last matmul. Hence:
  - the two tensors the first matmul needs (xh k=0, w k=0/h=0) are the FIRST
    triggers on two different queues (sync, scalar) so their ring cold-starts
    overlap;
  - activations alternate sync/vector so two rings stream them in parallel;
  - weights are batched into one trigger per h-phase (the per-trigger cost
    is ~600 ns of queue time) and sequenced h1 -> h2 -> h3 on gpsimd, each
    landing well before its compute phase;
  - the last (h,b2) group's elementwise tail is split into free-dim chunks
    so the post-matmul serial chain drains in pieces.
"""

import numpy as np

import concourse.bacc as bacc
import concourse.bass as bass
import concourse.mybir as mybir
from concourse import tile
from concourse.bass_utils import run_bass_kernel_spmd

N_CORES = 8
BATCH = 8192
B = BATCH // N_CORES  # 1024 batch rows per core
K = 1024              # fan_in = input_dim + hidden_dim
H = 512               # hidden dim
NG = 4                # gates: i, f, c, o
KT = K // 128         # 8 contraction tiles
HT = H // 128         # 4 hidden chunks per gate
BT = B // 512         # 2 batch halves (PSUM free-dim limit is 512 f32)

MM_DT = mybir.dt.bfloat16
BF16 = mybir.dt.bfloat16
F32 = mybir.dt.float32

_SIG = mybir.ActivationFunctionType.Sigmoid
_TANH = mybir.ActivationFunctionType.Tanh
# gate order within the concatenated weight: i, f, c, o
_GATE_FN = [_SIG, _SIG, _TANH, _SIG]


def _build():
    nc = bacc.Bacc(
        "TRN2",
        target_bir_lowering=False,
        debug=False,
        num_devices=N_CORES,
    )

    # Per-k bundle: activations k-tile (1024 cols) + the h=0 weight strip
    # for that k (512 gate-major cols), side by side, so one trigger per
    # k-tile delivers everything the h=0 phase needs for that k.
    xw = nc.dram_tensor("xw", [KT, 128, B + NG * 128], MM_DT, kind="ExternalInput")
    # Per-h weight block (h>=1) stored as its exact SBUF image: wH[h][p] is
    # the full 8 KB partition row (KT strips of NG*128 gate-major cols), so
    # each phase's load is one trigger with fully contiguous 8 KB elements.
    wH = nc.dram_tensor(
        "wH", [HT - 1, 128, KT, NG * 128], MM_DT, kind="ExternalInput")
    bias2d = nc.dram_tensor("bias2d", [128, NG * HT], F32, kind="ExternalInput")
    c_prevT = nc.dram_tensor("c_prevT", [128, HT, B], BF16, kind="ExternalInput")
    h_nextT = nc.dram_tensor("h_nextT", [H, B], BF16, kind="ExternalOutput")
    c_nextT = nc.dram_tensor("c_nextT", [H, B], BF16, kind="ExternalOutput")

    with tile.TileContext(nc) as tc:
        with (
            tc.tile_pool(name="wts", bufs=1) as wpool,
            tc.tile_pool(name="acts", bufs=1) as apool,
            tc.tile_pool(name="cprev", bufs=1) as cpool,
            tc.tile_pool(name="gates", bufs=3) as gpool,
            tc.tile_pool(name="ew", bufs=3) as epool,
            tc.tile_pool(name="psum", bufs=1, space="PSUM") as pspool,
        ):
            # --- PE warmup ----------------------------------------------
            # The PE runs matmuls ~2x slow until its high-activity boost
            # engages ~4.9 us after the first matmul (HAM record in the
            # trace). A few dummy matmuls on memset garbage, issued while
            # the input DMAs are still in flight, start that clock early
            # so the real stream runs boosted almost from its first op.
            wu_w = epool.tile([128, 128], MM_DT, tag="wu_w", name="wu_w")
            nc.vector.memset(wu_w[:], 0)
            wu_x = epool.tile([128, 512], MM_DT, tag="wu_x", name="wu_x")
            nc.vector.memset(wu_x[:], 0)
            wu_ps = pspool.tile([128, 512], F32, tag="ps3_1", name="wu_ps")
            for _ in range(4):
                nc.tensor.matmul(wu_ps[:], wu_w[:], wu_x[:], start=True, stop=True)

            # --- input DMA schedule -------------------------------------
            # Ring speeds measured from traces: sync (SP HWDGE) ~230 GB/s,
            # gpsimd ~200 GB/s, scalar (Activation HWDGE) ~45-130 GB/s.
            # The h=0-critical k-bundles therefore ride ONLY sync+gpsimd,
            # alternating; k=0 is split so the first matmul's data lands
            # as early as possible. Late-deadline loads (w_h1, c_prev,
            # w_h3) ride the gpsimd FIFO tail where they cannot preempt
            # the critical stream; the slow scalar ring gets only the tiny
            # bias and the mid-kernel w_h2. Output DMAs ride gpsimd after
            # its input transfers drain.
            # Bundle column layout: [acts b2=0 | h0 weights | acts b2=1].
            xw_tiles = [None] * KT

            xt0 = apool.tile([128, B + NG * 128], MM_DT, tag="xw0", name="xw0")
            nc.sync.dma_start(xt0[:, 0:1024], xw[0, :, 0:1024])
            nc.sync.dma_start(xt0[:, 1024:1536], xw[0, :, 1024:1536])
            xw_tiles[0] = xt0
            for k in range(1, KT):
                xt = apool.tile([128, B + NG * 128], MM_DT, tag=f"xw{k}", name=f"xw{k}")
                (nc.sync if k % 2 == 0 else nc.gpsimd).dma_start(xt[:], xw[k])
                xw_tiles[k] = xt

            bias_t = wpool.tile([128, NG * HT], F32, tag="bias", name="bias_t")
            nc.scalar.dma_start(bias_t[:], bias2d[:])

            wh_tiles = [None] * HT
            for h in range(1, HT):
                wh_tiles[h] = wpool.tile([128, KT, NG * 128], MM_DT,
                                         tag=f"w_h{h}", name=f"w_h{h}")

            # late-deadline loads behind the k-bundles (ring-FIFO ordered)
            cp_tile = cpool.tile([128, HT, B], BF16, tag="cp", name="cp")
            nc.gpsimd.dma_start(wh_tiles[1][:], wH[0])
            nc.gpsimd.dma_start(cp_tile[:], c_prevT[:])
            nc.gpsimd.dma_start(wh_tiles[3][:], wH[2])
            nc.scalar.dma_start(wh_tiles[2][:], wH[1])

            def _rhs(k, b2):
                off = 0 if b2 == 0 else B // 2 + NG * 128
                return xw_tiles[k][:, off:off + 512]

            def _lhsT(k, h, g):
                if h == 0:
                    return xw_tiles[k][:, 512 + g * 128:512 + (g + 1) * 128]
                return wh_tiles[h][:, k, g * 128:(g + 1) * 128]

            def _mk_psum(g, h, b2):
                return pspool.tile(
                    [128, 512], F32,
                    tag=f"ps{g}_{b2 % 2}", name=f"ps{g}_{h}_{b2}",
                )

            def _elementwise(h, b2, psum, chunks=1, dma_eng=None):
                """Activations + LSTM cell tail for one (h, b2) group.

                chunks>1 splits the free dim so the final group's serial
                ACT->DVE->ACT->DVE chain drains in smaller pieces.
                """
                dma_eng = dma_eng or nc.gpsimd
                hs = slice(h * 128, (h + 1) * 128)
                w = 512 // chunks

                def _act_gate(g, c):
                    t = gpool.tile(
                        [128, w], F32, tag=f"g{g}", name=f"g{g}_{h}_{b2}_{c}",
                    )
                    nc.scalar.activation(
                        t[:], psum[g][:, c * w:(c + 1) * w], _GATE_FN[g],
                        bias=bias_t[:, g * HT + h:g * HT + h + 1],
                    )
                    return t

                for c in range(chunks):
                    cs = slice(b2 * 512 + c * w, b2 * 512 + (c + 1) * w)
                    # i, f, c~ first; the whole c_next/tanh chain runs while
                    # the output gate's matmuls are still on the PE (gate-
                    # major issue order puts o last).
                    gi = _act_gate(0, c)
                    gf = _act_gate(1, c)
                    gc = _act_gate(2, c)

                    t1 = epool.tile([128, w], F32, tag="t1", name=f"t1_{h}_{b2}_{c}")
                    nc.vector.tensor_mul(t1[:], gi[:], gc[:])       # i * c~
                    t2 = epool.tile([128, w], F32, tag="t2", name=f"t2_{h}_{b2}_{c}")
                    nc.vector.tensor_mul(t2[:], gf[:], cp_tile[:, h, cs])
                    cn = epool.tile([128, w], BF16, tag="cn", name=f"cn_{h}_{b2}_{c}")
                    nc.vector.tensor_add(cn[:], t1[:], t2[:])
                    dma_eng.dma_start(c_nextT[hs, cs], cn[:])

                    th = epool.tile([128, w], F32, tag="th", name=f"th_{h}_{b2}_{c}")
                    nc.scalar.activation(th[:], cn[:], _TANH)

                    go = _act_gate(3, c)
                    hn = epool.tile([128, w], BF16, tag="hn", name=f"hn_{h}_{b2}_{c}")
                    nc.vector.tensor_mul(hn[:], go[:], th[:])
                    dma_eng.dma_start(h_nextT[hs, cs], hn[:])

            # h=0 rides the input-DMA ramp: every group needs all 8 k-tiles,
            # so widen to all 8 PSUM banks (4 gates x 2 batch halves) and
            # issue k-major -- the PE consumes each k-tile pair 8 matmuls at
            # a time, right as it lands.
            psum0 = {b2: [_mk_psum(g, 0, b2) for g in range(NG)] for b2 in range(BT)}
            for k in range(KT):
                for g in range(NG):
                    for b2 in range(BT):
                        nc.tensor.matmul(
                            psum0[b2][g][:],
                            _lhsT(k, 0, g),
                            _rhs(k, b2),
                            start=(k == 0),
                            stop=(k == KT - 1),
                        )
            for b2 in range(BT):
                _elementwise(0, b2, psum0[b2])

            # h>=1: inputs are resident; per-(h,b2) 4-bank groups with b2
            # parity alternating between the two bank sets, so each set's
            # ACT drain overlaps the other's matmuls.
            for h in range(1, HT):
                for b2 in range(BT):
                    psum = [_mk_psum(g, h, b2) for g in range(NG)]
                    # gate-major, output gate (g=3) last: everything except
                    # ACT(o) and h=o*tanh(c) drains while o's matmuls run.
                    for g in range(NG):
                        for k in range(KT):
                            nc.tensor.matmul(
                                psum[g][:],
                                _lhsT(k, h, g),
                                _rhs(k, b2),
                                start=(k == 0),
                                stop=(k == KT - 1),
                            )
                    last = (h == HT - 1 and b2 == BT - 1)
                    _elementwise(
                        h, b2, psum,
                        chunks=2 if last else 1,
                        dma_eng=nc.sync if last else None,
                    )

    nc.compile()
    return nc


_NC_CACHE = None
_LAST_IN_MAPS = None


def kernel(x, h_prev, c_prev, W_i, b_i, W_f, b_f, W_c, b_c, W_o, b_o):
    global _NC_CACHE, _LAST_IN_MAPS
    if _NC_CACHE is None:
        _NC_CACHE = _build()
    nc = _NC_CACHE

    np_bf16 = mybir.dt.np(MM_DT)

    combT = np.concatenate([x, h_prev], axis=1).T          # (K, BATCH) f32
    combT = combT.astype(np_bf16)
    wT = np.concatenate([W_i, W_f, W_c, W_o], axis=0).T    # (K, 4H): col g*H+h*128+p
    # wHk[h][p][k] = 512 gate-major cols of strip (k, h) for partition p
    wHk = np.ascontiguousarray(
        wT.reshape(KT, 128, NG, HT, 128).transpose(3, 1, 0, 2, 4)
        .reshape(HT, 128, KT, NG * 128)
    ).astype(np_bf16)
    bias2d = np.ascontiguousarray(
        np.concatenate([b_i, b_f, b_c, b_o]).reshape(NG * HT, 128).T
    ).astype(np.float32)                                   # (128, 16)
    c_prevT = c_prev.T.astype(np_bf16)                     # (H, BATCH) bf16

    in_maps = []
    for j in range(N_CORES):
        cols = slice(j * B, (j + 1) * B)
        # per-k bundle: [acts b2=0 | h=0 weight strip | acts b2=1]
        acts = combT[:, cols].reshape(KT, 128, B)
        xwj = np.concatenate(
            [acts[:, :, :512], wHk[0].transpose(1, 0, 2), acts[:, :, 512:]],
            axis=2)
        in_maps.append({
            "xw": np.ascontiguousarray(xwj),
            "wH": wHk[1:],
            "bias2d": bias2d,
            "c_prevT": np.ascontiguousarray(
                c_prevT[:, cols].reshape(HT, 128, B).transpose(1, 0, 2)
            ),
        })

    _LAST_IN_MAPS = in_maps
    try:
        res = run_bass_kernel_spmd(nc, in_maps, core_ids=list(range(N_CORES)))
    except Exception:
        # transient NRT_EXEC_UNIT_UNRECOVERABLE has been observed once on an
        # otherwise-correct NEFF; one retry is cheap insurance.
        res = run_bass_kernel_spmd(nc, in_maps, core_ids=list(range(N_CORES)))

    h_next = np.concatenate(
        [r["h_nextT"].astype(np.float32).T for r in res.results], axis=0)
    c_next = np.concatenate(
        [r["c_nextT"].astype(np.float32).T for r in res.results], axis=0)
    return (h_next, c_next)
